# revision 1
# baseline (speedup 1.0000x reference)
"""Trainium2 Bass kernel for nn_BasicNCAModel (neural cellular automaton).

Model (per step, 4 steps):
  y = concat([x, dwconv3x3(x, f1), dwconv3x3(x, f2)])   (reflect pad)
  dx = relu(y @ w1 + b1) @ w2
  x  = x + dx * (stoch > 0.5) * ch_mask

Kernel strategy:
  - Pure data parallel: batch 16 -> 2 samples on each of 8 NeuronCores.
  - Channel-major layout [C=32, H, W]; the depthwise convs + first dense
    layer fold into a single 3x3 conv with effective weights
    W_eff[dy,dx] = diag(f1[dy,dx]) @ w1[32:64] + diag(f2[dy,dx]) @ w1[64:96]
    (+ w1[0:32] at the center tap). Per 512-pixel tile this is 6 matmuls
    (3 horizontal taps x 2 output halves of 256) with K=96 = 3 vertically
    shifted copies of x stacked on partitions; horizontal taps are free-dim
    AP offsets. The bias rides as a 97th ones-row on the center tap.
    Second layer: 2 matmuls K=128. ch_mask is folded into w2 (cols 0..2
    zeroed), so the residual add is exact for the image channels.
  - Matmul operands are fp16 (full PE rate + fast weight load; ~2^-11
    rounding like fp32r but without its half-rate 2-pass behavior).
    The residual add x + dx*mask runs in exact fp32 on the vector engine
    from a separate fp32 load of the band interior.
  - x lives in DRAM column-padded [C, H, W+2] so band loads/stores are
    fully contiguous per partition; reflect rows are handled by DMA
    segmenting, reflect columns by two tiny on-chip copies. x ping-pongs
    between two internal DRAM buffers across the 4 steps.
"""

import numpy as np
from contextlib import ExitStack

import concourse.bacc as bacc
import concourse.tile as tile
from concourse import mybir
from concourse.bass_utils import run_bass_kernel_spmd

F32 = mybir.dt.float32
F16 = mybir.dt.float16
AF = mybir.ActivationFunctionType
OP = mybir.AluOpType

B, C, H, W = 16, 32, 256, 256
IMG = 3
FIRE = 0.5
NCORES = 8
BPC = B // NCORES          # samples per core = 2
BR = 16                    # band rows
NB = H // BR               # bands per sample = 16
ROWS_PER_TILE = 2          # 2 rows x 256 cols = 512-pixel matmul tiles
TPB = BR // ROWS_PER_TILE  # tiles per band = 8
NSTEP = 4
WP = W + 2                 # padded row length 258


def _seg_rows(r0: int, dy: int):
    """Contiguous (src_row, dst_row, n) segments for one vertical copy,
    with reflect handling at the image top/bottom (reflect: -1->1, 256->254)."""
    rows = [r0 + dy + i for i in range(BR)]
    refl = [(-r if r < 0 else (2 * (H - 1) - r if r > H - 1 else r)) for r in rows]
    segs = []
    i = 0
    while i < BR:
        j = i + 1
        while j < BR and refl[j] == refl[i] + (j - i):
            j += 1
        segs.append((refl[i], i, j - i))
        i = j
    return segs


def _build():
    nc = bacc.Bacc("TRN2", target_bir_lowering=False, debug=False,
                   num_devices=NCORES)
    xin = nc.dram_tensor("xin", [BPC, C, H, WP], F32, kind="ExternalInput").ap()
    stoch = nc.dram_tensor("stoch", [NSTEP, BPC, H, W], F32,
                           kind="ExternalInput").ap()
    wm = nc.dram_tensor("wm", [96, 256], F16, kind="ExternalInput").ap()
    w0 = nc.dram_tensor("w0", [97, 256], F16, kind="ExternalInput").ap()
    wp = nc.dram_tensor("wp", [96, 256], F16, kind="ExternalInput").ap()
    w2h = nc.dram_tensor("w2h", [128, 64], F16, kind="ExternalInput").ap()
    yout = nc.dram_tensor("y", [BPC, C, H, WP], F32, kind="ExternalOutput").ap()

    with tile.TileContext(nc) as tc, ExitStack() as ctx:
        dram = ctx.enter_context(tc.tile_pool(name="dram", bufs=1, space="DRAM"))
        xA = dram.tile([BPC, C, H, WP], F32, name="xA")
        xB = dram.tile([BPC, C, H, WP], F32, name="xB")

        wpool = ctx.enter_context(tc.tile_pool(name="wpool", bufs=1))
        wmt = wpool.tile([96, 256], F16, name="wmt")
        w0t = wpool.tile([97, 256], F16, name="w0t")
        wpt = wpool.tile([96, 256], F16, name="wpt")
        w2t = wpool.tile([128, 64], F16, name="w2t")
        ones = wpool.tile([1, BR * WP], F16, name="ones")
        nc.sync.dma_start(wmt[:], wm)
        nc.sync.dma_start(w0t[:], w0)
        nc.sync.dma_start(wpt[:], wp)
        nc.sync.dma_start(w2t[:], w2h)
        nc.gpsimd.memset(ones[:], 1.0)

        xt_pool = ctx.enter_context(tc.tile_pool(name="xt", bufs=4))
        xc_pool = ctx.enter_context(tc.tile_pool(name="xc", bufs=2))
        st_pool = ctx.enter_context(tc.tile_pool(name="st", bufs=2))
        stb_pool = ctx.enter_context(tc.tile_pool(name="stb", bufs=2))
        xn_pool = ctx.enter_context(tc.tile_pool(name="xn", bufs=2))
        hs_pool = ctx.enter_context(tc.tile_pool(name="hs", bufs=3))
        dxm_pool = ctx.enter_context(tc.tile_pool(name="dxm", bufs=3))
        hp_pool = ctx.enter_context(tc.tile_pool(name="hp", bufs=3, space="PSUM"))
        dxp_pool = ctx.enter_context(tc.tile_pool(name="dxp", bufs=2, space="PSUM"))

        srcs = [xin, xA[:], xB[:], xA[:]]
        dsts = [xA[:], xB[:], xA[:], yout]

        for step in range(NSTEP):
            src, dst = srcs[step], dsts[step]
            for s in range(BPC):
                for b in range(NB):
                    r0 = b * BR
                    # ---- load: 3 vertically shifted fp16 copies of the band.
                    # partition groups: 0-31 dy=0 (center), 32-63 dy=-1,
                    # 64-95 dy=+1 — center first so the residual/mask ops all
                    # share base partition 0 (DVE needs equal base partitions).
                    xt = xt_pool.tile([97, BR * WP], F16)
                    xtr = xt[:].rearrange("p (r c) -> p r c", c=WP)
                    for gi, dy in enumerate((0, -1, 1)):
                        p0 = gi * 32
                        for (sr, dr, n) in _seg_rows(r0, dy):
                            # SWDGE load, cast f32 -> fp16 in flight (cheap
                            # trigger; descriptor gen runs on Q7 cores, off
                            # the engine queues); contiguous [n*WP]/channel
                            nc.gpsimd.dma_start(
                                xtr[p0:p0 + 32, dr:dr + n, :],
                                src[s, :, sr:sr + n, :])
                    # ones row for the bias (97th K row of the center tap)
                    nc.gpsimd.dma_start(xt[96:97, :], ones[:])
                    # reflect column pads: col0 <- col2, col257 <- col255
                    nc.vector.tensor_copy(xtr[0:96, :, 0:1], xtr[0:96, :, 2:3])
                    nc.vector.tensor_copy(xtr[0:96, :, WP - 1:WP],
                                          xtr[0:96, :, WP - 3:WP - 2])

                    # exact fp32 copy of the band for the residual add
                    xc = xc_pool.tile([32, BR * WP], F32)
                    xcr = xc[:].rearrange("p (r c) -> p r c", c=WP)
                    nc.gpsimd.dma_start(xc[:], src[s, :, r0:r0 + BR, :]
                                        .rearrange("p r c -> p (r c)"))

                    # ---- stochastic values, broadcast across channels ----
                    st = st_pool.tile([1, BR * W], F32)
                    nc.gpsimd.dma_start(
                        st[:], stoch[step, s, r0:r0 + BR, :].flatten().unsqueeze(0))
                    stb = stb_pool.tile([32, BR * W], F32)
                    nc.gpsimd.partition_broadcast(stb[:], st[:])
                    stbr = stb[:].rearrange("p (r c) -> p r c", c=W)

                    xn = xn_pool.tile([32, BR * WP], F32)
                    xnr = xn[:].rearrange("p (r c) -> p r c", c=WP)
                    # pad columns are stored to DRAM but never consumed as
                    # data; init them so the contiguous store reads defined
                    # memory (single strided memset covers cols 0 and 257)
                    nc.gpsimd.memset(xnr[:, :, 0:WP:WP - 1], 0.0)

                    # software pipeline: layer 2 of tile t-1 is emitted after
                    # layer 1 of tile t, giving the relu a full L1-block of
                    # slack before the PE needs its output
                    pend = None  # (hs, rt) awaiting layer 2
                    for t in range(TPB + 1):
                        if t < TPB:
                            rt = t * ROWS_PER_TILE
                            # ---- layer 1: 3x3 conv (3 taps x 2 halves) ----
                            hp = hp_pool.tile([128, 1024], F32)
                            for h in range(2):
                                out = hp[:, h * 512:(h + 1) * 512]
                                hslc = slice(h * 128, (h + 1) * 128)
                                nc.tensor.matmul(
                                    out, wmt[:, hslc],
                                    xtr[0:96, rt:rt + ROWS_PER_TILE, 0:W],
                                    start=True, stop=False)
                                nc.tensor.matmul(
                                    out, w0t[:, hslc],
                                    xtr[0:97, rt:rt + ROWS_PER_TILE, 1:W + 1],
                                    start=False, stop=False)
                                nc.tensor.matmul(
                                    out, wpt[:, hslc],
                                    xtr[0:96, rt:rt + ROWS_PER_TILE, 2:W + 2],
                                    start=False, stop=True)
                            # ---- relu (bias already added via ones row) ----
                            hs = hs_pool.tile([128, 1024], F16)
                            nc.scalar.activation(hs[:], hp[:], AF.Relu)
                            pend, prev = (hs, rt), pend
                        else:
                            prev, pend = pend, None
                        if prev is None:
                            continue
                        hs_p, rp = prev
                        # ---- layer 2: dx = h @ w2 (K=256 split in two) ----
                        dxp = dxp_pool.tile([32, 512], F32)
                        nc.tensor.matmul(dxp[:], w2t[:, 0:32], hs_p[:, 0:512],
                                         start=True, stop=False)
                        nc.tensor.matmul(dxp[:], w2t[:, 32:64], hs_p[:, 512:1024],
                                         start=False, stop=True)
                        # ---- masked residual: xn = (stoch>0.5)*dx + x ----
                        dxpr = dxp[:].rearrange("p (r c) -> p r c", c=W)
                        dxm = dxm_pool.tile([32, ROWS_PER_TILE * W], F32)
                        dxmr = dxm[:].rearrange("p (r c) -> p r c", c=W)
                        nc.vector.scalar_tensor_tensor(
                            dxmr, stbr[:, rp:rp + ROWS_PER_TILE, :], FIRE,
                            dxpr, op0=OP.is_gt, op1=OP.mult)
                        nc.vector.tensor_add(
                            xnr[:, rp:rp + ROWS_PER_TILE, 1:W + 1], dxmr,
                            xcr[:, rp:rp + ROWS_PER_TILE, 1:W + 1])

                    # ---- store band (contiguous, pads included) ----
                    nc.sync.dma_start(
                        dst[s, :, r0:r0 + BR, :].rearrange("p r c -> p (r c)"),
                        xn[:])
    nc.compile()
    return nc


_NC_CACHE = None


def _get_nc():
    global _NC_CACHE
    if _NC_CACHE is None:
        _NC_CACHE = _build()
    return _NC_CACHE


def _make_in_maps(x, f1, f2, w1, b1, w2, stoch):
    f1 = np.asarray(f1, np.float64)[:, :, 0, :]   # [3,3,32]
    f2 = np.asarray(f2, np.float64)[:, :, 0, :]
    w1 = np.asarray(w1, np.float64)               # [96,256]
    b1 = np.asarray(b1, np.float64)               # [256]
    w2 = np.asarray(w2, np.float64).copy()        # [256,32]
    w2[:, :IMG] = 0.0                             # ch_mask folded into w2

    # W_eff[dy,dx][c,:] = f1[dy,dx,c]*w1[32+c,:] + f2[dy,dx,c]*w1[64+c,:]
    #                     (+ w1[c,:] at the center tap)
    weff = (f1[:, :, :, None] * w1[None, None, 32:64, :]
            + f2[:, :, :, None] * w1[None, None, 64:96, :])   # [3,3,32,256]
    weff[1, 1] += w1[0:32, :]

    def col(dxi):  # stack the 3 vertical taps along K for horizontal tap dxi
        # row order matches xt partition groups: dy=0, dy=-1, dy=+1
        return np.concatenate([weff[1, dxi], weff[0, dxi], weff[2, dxi]], axis=0)

    wm = col(0).astype(np.float16)                                    # [96,256]
    w0 = np.concatenate([col(1), b1[None, :]], axis=0).astype(np.float16)
    wpm = col(2).astype(np.float16)                                   # [96,256]
    w2h = np.concatenate([w2[0:128, :], w2[128:256, :]], axis=1).astype(np.float16)

    x = np.asarray(x, np.float32)
    stoch = np.asarray(stoch, np.float32)
    in_maps = []
    for i in range(NCORES):
        xi = np.transpose(x[i * BPC:(i + 1) * BPC], (0, 3, 1, 2))  # [2,32,H,W]
        xpad = np.zeros((BPC, C, H, WP), np.float32)
        xpad[:, :, :, 1:W + 1] = xi
        sti = np.ascontiguousarray(
            stoch[:, i * BPC:(i + 1) * BPC, :, :, 0])
        in_maps.append({"xin": xpad, "stoch": sti, "wm": wm, "w0": w0,
                        "wp": wpm, "w2h": w2h})
    return in_maps


def kernel(x, f1, f2, w1, b1, w2, stoch, steps):
    assert int(steps) == NSTEP, f"kernel compiled for {NSTEP} steps, got {steps}"
    nc = _get_nc()
    in_maps = _make_in_maps(x, f1, f2, w1, b1, w2, stoch)
    res = run_bass_kernel_spmd(nc, in_maps, core_ids=list(range(NCORES)))
    outs = []
    for i in range(NCORES):
        yi = res.results[i]["y"][:, :, :, 1:W + 1]     # strip col pads
        outs.append(np.transpose(yi, (0, 2, 3, 1)))    # -> [2,256,256,32]
    return np.ascontiguousarray(np.concatenate(outs, axis=0)).astype(np.float32)



# revision 17
# speedup vs baseline: 1.7786x; 1.7786x over previous
"""Trainium2 Bass kernel for nn_BasicNCAModel (neural cellular automaton).

Model (per step, 4 steps):
  y = concat([x, dwconv3x3(x, f1), dwconv3x3(x, f2)])   (reflect pad)
  dx = relu(y @ w1 + b1) @ w2
  x  = x + dx * (stoch > 0.5) * ch_mask

Kernel strategy:
  - Pure data parallel: batch 16 -> 2 samples on each of 8 NeuronCores.
  - Channel-major layout [C=32, H, W]; depthwise convs + first dense layer
    fold into a 3x3 conv with effective weights
    W_eff[dy,dx] = diag(f1[dy,dx]) @ w1[32:64] + diag(f2[dy,dx]) @ w1[64:96]
    (+ w1[0:32] at the center tap). K = 3 vertically shifted copies of x
    stacked on partitions; horizontal taps are free-dim AP offsets.
  - Mixed precision: the two outer horizontal taps (only ~15% of the dx
    variance) form the 2 k-tiles of ONE fp8e4 DoubleRow matmul per output
    half (overlapping stride-2 moving AP over an fp8 copy of the band);
    the center tap (K=97 incl. ones row for the bias) and layer 2 run in
    fp16 to keep quantization error ~1e-2. 6 PE passes of 512 moving rows
    per 512-pixel tile (vs 8 all-fp16 passes).
  - The fire mask is precomputed 0/1 fp8 on host [H, W] and replicated 32x
    on device into [H/2, 32, 2, W] via DRAM->DRAM DMAs; its (row-pair,
    channel) -> partition map is affine so one HWDGE DMA loads a whole
    band's mask [32, BR*W] with no on-chip broadcast.
  - Relu+bias splits between the scalar engine (cols [0:RELU_ACT]) and DVE
    (tensor_scalar_max, cols [RELU_ACT:1024]), both writing fp8.
  - Residual: dxm = mask * dx on DVE (PSUM source) into a band-level fp16
    tile. Then either
      RESID="accum": the state never passes through compute engines - the
        old band is DRAM->DRAM precopied to dst and dxm is accumulated
        into it by a SWDGE read-modify-write DMA (interior + pad columns);
      RESID="add": classic xc load + DVE fp16 add + store.
  - State x is fp16 DRAM, column-padded [C, H, W+2] with reflect columns
    stored in the pads (band loads contiguous). Ping-pong across steps.
"""

import numpy as np
import ml_dtypes
from contextlib import ExitStack

import concourse.bacc as bacc
import concourse.tile as tile
from concourse import mybir
from concourse.ap import AP
from concourse.bass_utils import run_bass_kernel_spmd

F32 = mybir.dt.float32
F16 = mybir.dt.float16
F8 = mybir.dt.float8e4
AF = mybir.ActivationFunctionType
OP = mybir.AluOpType
DRMODE = mybir.MatmulPerfMode.DoubleRow

B, C, H, W = 16, 32, 256, 256
IMG = 3
FIRE = 0.5
NCORES = 8
BPC = B // NCORES          # samples per core = 2
BR = 16                    # band rows
NB = H // BR               # bands per sample = 16
ROWS_PER_TILE = 2          # 2 rows x 256 cols = 512-pixel matmul tiles
TPB = BR // ROWS_PER_TILE  # tiles per band = 8
NSTEP = 4
WP = W + 2                 # padded row length 258

RESID = "accum"            # "accum" (SWDGE RMW store) or "add" (DVE add)
RELU_ACT = 704             # relu split: scalar engine cols [0:RELU_ACT]


def _seg_rows(r0: int, dy: int):
    """Contiguous (src_row, dst_row, n) segments for one vertical copy,
    with reflect handling at the image top/bottom (reflect: -1->1, 256->254)."""
    rows = [r0 + dy + i for i in range(BR)]
    refl = [(-r if r < 0 else (2 * (H - 1) - r if r > H - 1 else r)) for r in rows]
    segs = []
    i = 0
    while i < BR:
        j = i + 1
        while j < BR and refl[j] == refl[i] + (j - i):
            j += 1
        segs.append((refl[i], i, j - i))
        i = j
    return segs


def _build():
    nc = bacc.Bacc("TRN2", target_bir_lowering=False, debug=False,
                   num_devices=NCORES)
    xin = nc.dram_tensor("xin", [BPC, C, H, WP], F16, kind="ExternalInput").ap()
    mask8 = nc.dram_tensor("mask8", [NSTEP, BPC, H, W], F8,
                           kind="ExternalInput").ap()
    wmp = nc.dram_tensor("wmp", [96, 512], F8, kind="ExternalInput").ap()
    w0 = nc.dram_tensor("w0", [97, 256], F16, kind="ExternalInput").ap()
    w2h = nc.dram_tensor("w2h", [128, 64], F16, kind="ExternalInput").ap()
    yout = nc.dram_tensor("y", [BPC, C, H, WP], F16, kind="ExternalOutput").ap()

    with tile.TileContext(nc) as tc, ExitStack() as ctx:
        dram = ctx.enter_context(tc.tile_pool(name="dram", bufs=1, space="DRAM"))
        xA = dram.tile([BPC, C, H, WP], F16, name="xA")
        xB = dram.tile([BPC, C, H, WP], F16, name="xB")
        # mask replicated 32x: [step, s, row-pair, channel-copy, 2, W]
        mrep = dram.tile([NSTEP, BPC, H // 2, 32, 2, W], F8, name="mrep")

        # ---- replicate the compact mask to all 32 channel slots ----
        for step in range(NSTEP):
            for s in range(BPC):
                msrc = mask8[step, s].rearrange("(p two) w -> p two w", two=2)
                for c in range(32):
                    nc.sync.dma_start(mrep[step, s, :, c], msrc)

        wpool = ctx.enter_context(tc.tile_pool(name="wpool", bufs=1))
        wmpt = wpool.tile([96, 512], F8, name="wmpt")
        w0t = wpool.tile([97, 256], F16, name="w0t")
        w2t = wpool.tile([128, 64], F16, name="w2t")
        nc.sync.dma_start(wmpt[:], wmp)
        nc.sync.dma_start(w0t[:], w0)
        nc.sync.dma_start(w2t[:], w2h)

        xt_pool = ctx.enter_context(tc.tile_pool(name="xt", bufs=4))
        ms_pool = ctx.enter_context(tc.tile_pool(name="ms", bufs=2))
        dxm_pool = ctx.enter_context(tc.tile_pool(name="dxm", bufs=2))
        hs_pool = ctx.enter_context(tc.tile_pool(name="hs", bufs=3))
        hp_pool = ctx.enter_context(tc.tile_pool(name="hp", bufs=3, space="PSUM"))
        dxp_pool = ctx.enter_context(tc.tile_pool(name="dxp", bufs=2, space="PSUM"))
        if RESID == "add":
            xc_pool = ctx.enter_context(tc.tile_pool(name="xc", bufs=2))
            xn_pool = ctx.enter_context(tc.tile_pool(name="xn", bufs=2))

        # fp8 copy of the band for the DoubleRow outer-tap pass
        xt8_pool = ctx.enter_context(tc.tile_pool(name="xt8", bufs=4))
        # manual 4-buffer rotation for the fp16 copy so the ones row (bias)
        # is primed once per buffer instead of per band
        xts = [xt_pool.tile([97, BR * WP], F16, name=f"xt{i}")
               for i in range(4)]
        for xt in xts:
            nc.gpsimd.memset(xt[96:97, :], 1.0)

        srcs = [xin, xA[:], xB[:], xA[:]]
        dsts = [xA[:], xB[:], xA[:], yout]

        band_idx = 0
        for step in range(NSTEP):
            src, dst = srcs[step], dsts[step]
            for s in range(BPC):
                for b in range(NB):
                    r0 = b * BR
                    dst_band = dst[s, :, r0:r0 + BR, :]
                    if RESID == "accum":
                        # precopy old state band into dst (DRAM->DRAM);
                        # dxm will be accumulated into it at band end
                        nc.sync.dma_start(
                            dst_band.rearrange("p r c -> p (r c)"),
                            src[s, :, r0:r0 + BR, :]
                            .rearrange("p r c -> p (r c)"))

                    # ---- load: 3 vertically shifted copies of the band.
                    # partition groups: 0-31 dy=0, 32-63 dy=-1, 64-95 dy=+1.
                    # fp16 copy via HWDGE (center tap); fp8 copy via SWDGE
                    # cast in flight (DoubleRow outer taps). Reflect columns
                    # are already stored in the DRAM pads.
                    xt = xts[band_idx % 4]
                    band_idx += 1
                    xtr = xt[:].rearrange("p (r c) -> p r c", c=WP)
                    xt8 = xt8_pool.tile([96, BR * WP], F8)
                    xt8r = xt8[:].rearrange("p (r c) -> p r c", c=WP)
                    for gi, dy in enumerate((0, -1, 1)):
                        p0 = gi * 32
                        for (sr, dr, n) in _seg_rows(r0, dy):
                            nc.sync.dma_start(
                                xtr[p0:p0 + 32, dr:dr + n, :],
                                src[s, :, sr:sr + n, :])
                            nc.gpsimd.dma_start(
                                xt8r[p0:p0 + 32, dr:dr + n, :],
                                src[s, :, sr:sr + n, :])

                    # ---- band fire mask [32, BR*W] via one affine DMA ----
                    ms = ms_pool.tile([32, BR * W], F8)
                    rp0 = r0 // 2
                    nc.sync.dma_start(
                        ms[:], mrep[step, s, rp0:rp0 + TPB]
                        .rearrange("a b c d -> b a (c d)"))

                    # band-level masked-update tile (written per tile)
                    dxm = dxm_pool.tile([32, BR * W], F16)

                    if RESID == "add":
                        xc = xc_pool.tile([32, BR * WP], F16)
                        xcr = xc[:].rearrange("p (r c) -> p r c", c=WP)
                        nc.sync.dma_start(xc[:], src[s, :, r0:r0 + BR, :]
                                          .rearrange("p r c -> p (r c)"))
                        xn = xn_pool.tile([32, BR * WP], F16)
                        xnr = xn[:].rearrange("p (r c) -> p r c", c=WP)

                    xbase = xt8[:]
                    pstride = xbase.ap[0][0]

                    # software pipeline: layer 2 + mask of tile t-1 are
                    # emitted after layer 1 + relu of tile t
                    pend = None  # (hs, t) awaiting layer 2
                    for t in range(TPB + 1):
                        if t < TPB:
                            rt = t * ROWS_PER_TILE
                            # ---- layer 1: DR (taps -1,+1) + center, x2 ----
                            hp = hp_pool.tile([128, 1024], F32)
                            rhs_mp = AP(
                                tensor=xbase.tensor,
                                offset=xbase.offset + rt * WP,
                                ap=[[pstride, 96], [2, 2],
                                    [WP, ROWS_PER_TILE], [1, W]])
                            for h in range(2):
                                out = hp[:, h * 512:(h + 1) * 512]
                                lhs_mp = (wmpt[:, h * 256:(h + 1) * 256]
                                          .rearrange("p (i m) -> p i m", i=2))
                                nc.tensor.matmul(out, lhs_mp, rhs_mp,
                                                 start=True, stop=False,
                                                 perf_mode=DRMODE)
                                nc.tensor.matmul(
                                    out, w0t[:, h * 128:(h + 1) * 128],
                                    xtr[0:97, rt:rt + ROWS_PER_TILE, 1:W + 1],
                                    start=False, stop=True)
                            # ---- relu (bias via ones row) -> fp16, split
                            # between scalar engine and DVE ----
                            hs = hs_pool.tile([128, 1024], F16)
                            nc.scalar.activation(hs[:, 0:RELU_ACT],
                                                 hp[:, 0:RELU_ACT], AF.Relu)
                            nc.vector.tensor_scalar_max(
                                hs[:, RELU_ACT:1024], hp[:, RELU_ACT:1024],
                                0.0)
                            pend, prev = (hs, t), pend
                        else:
                            prev, pend = pend, None
                        if prev is None:
                            continue
                        hs_p, tp = prev
                        # ---- layer 2: dx = h @ w2, 2 fp16 matmuls ----
                        dxp = dxp_pool.tile([32, 512], F32)
                        nc.tensor.matmul(dxp[:], w2t[:, 0:32],
                                         hs_p[:, 0:512],
                                         start=True, stop=False)
                        nc.tensor.matmul(dxp[:], w2t[:, 32:64],
                                         hs_p[:, 512:1024],
                                         start=False, stop=True)
                        # ---- dxm = mask * dx (fp16) ----
                        csl = slice(tp * 512, (tp + 1) * 512)
                        nc.vector.tensor_tensor(dxm[:, csl], ms[:, csl],
                                                dxp[:], op=OP.mult)
                        if RESID == "add":
                            rp = tp * ROWS_PER_TILE
                            nc.vector.tensor_add(
                                xnr[:, rp:rp + ROWS_PER_TILE, 1:W + 1],
                                dxm[:, csl].rearrange("p (r c) -> p r c", c=W),
                                xcr[:, rp:rp + ROWS_PER_TILE, 1:W + 1])

                    dxmr = dxm[:].rearrange("p (r c) -> p r c", c=W)
                    if RESID == "accum":
                        # interior: dst[., r, 1+w] += dxm[., r, w]
                        nc.gpsimd.dma_start(
                            dst_band[:, :, 1:W + 1], dxmr, accum_op=OP.add)
                        # reflect pads: dst col0 += dxm col1; col257 += col254
                        nc.gpsimd.dma_start(
                            dst_band[:, :, 0:1].rearrange("p r o -> p (r o)"),
                            dxmr[:, :, 1:2].rearrange("p r o -> p (r o)"),
                            accum_op=OP.add)
                        nc.gpsimd.dma_start(
                            dst_band[:, :, WP - 1:WP]
                            .rearrange("p r o -> p (r o)"),
                            dxmr[:, :, 254:255].rearrange("p r o -> p (r o)"),
                            accum_op=OP.add)
                    else:
                        # reflect pads then store the fp16 band
                        nc.vector.tensor_copy(xnr[:, :, 0:1], xnr[:, :, 2:3])
                        nc.vector.tensor_copy(xnr[:, :, WP - 1:WP],
                                              xnr[:, :, WP - 3:WP - 2])
                        nc.sync.dma_start(
                            dst_band.rearrange("p r c -> p (r c)"), xn[:])
    nc.compile()
    return nc


_NC_CACHE = None


def _get_nc():
    global _NC_CACHE
    if _NC_CACHE is None:
        _NC_CACHE = _build()
    return _NC_CACHE


def _make_in_maps(x, f1, f2, w1, b1, w2, stoch):
    f1 = np.asarray(f1, np.float64)[:, :, 0, :]   # [3,3,32]
    f2 = np.asarray(f2, np.float64)[:, :, 0, :]
    w1 = np.asarray(w1, np.float64)               # [96,256]
    b1 = np.asarray(b1, np.float64)               # [256]
    w2 = np.asarray(w2, np.float64).copy()        # [256,32]
    w2[:, :IMG] = 0.0                             # ch_mask folded into w2

    # W_eff[dy,dx][c,:] = f1[dy,dx,c]*w1[32+c,:] + f2[dy,dx,c]*w1[64+c,:]
    #                     (+ w1[c,:] at the center tap)
    weff = (f1[:, :, :, None] * w1[None, None, 32:64, :]
            + f2[:, :, :, None] * w1[None, None, 64:96, :])   # [3,3,32,256]
    weff[1, 1] += w1[0:32, :]

    def col(dxi):  # stack the 3 vertical taps along K for horizontal tap dxi
        # row order matches xt partition groups: dy=0, dy=-1, dy=+1
        return np.concatenate([weff[1, dxi], weff[0, dxi], weff[2, dxi]], axis=0)

    F8NP = ml_dtypes.float8_e4m3
    wm, wpm = col(0), col(2)
    # DR stationary per half: [wm_half | wp_half] along the free dim
    wmp = np.concatenate([wm[:, 0:128], wpm[:, 0:128],
                          wm[:, 128:256], wpm[:, 128:256]], axis=1).astype(F8NP)
    w0 = np.concatenate([col(1), b1[None, :]], axis=0).astype(np.float16)
    w2h = np.concatenate([w2[0:128, :], w2[128:256, :]],
                         axis=1).astype(np.float16)

    x = np.asarray(x, np.float32)
    m8 = (np.asarray(stoch, np.float64) > FIRE).astype(F8NP)
    in_maps = []
    for i in range(NCORES):
        xi = np.transpose(x[i * BPC:(i + 1) * BPC], (0, 3, 1, 2))  # [2,32,H,W]
        xpad = np.zeros((BPC, C, H, WP), np.float16)
        xpad[:, :, :, 1:W + 1] = xi
        xpad[:, :, :, 0] = xi[:, :, :, 1]        # reflect col pads
        xpad[:, :, :, WP - 1] = xi[:, :, :, W - 2]
        mi = np.ascontiguousarray(m8[:, i * BPC:(i + 1) * BPC, :, :, 0])
        in_maps.append({"xin": xpad, "mask8": mi, "wmp": wmp, "w0": w0,
                        "w2h": w2h})
    return in_maps


def kernel(x, f1, f2, w1, b1, w2, stoch, steps):
    assert int(steps) == NSTEP, f"kernel compiled for {NSTEP} steps, got {steps}"
    nc = _get_nc()
    in_maps = _make_in_maps(x, f1, f2, w1, b1, w2, stoch)
    res = run_bass_kernel_spmd(nc, in_maps, core_ids=list(range(NCORES)))
    outs = []
    for i in range(NCORES):
        yi = res.results[i]["y"][:, :, :, 1:W + 1]     # strip col pads
        outs.append(np.transpose(yi, (0, 2, 3, 1)))    # -> [2,256,256,32]
    return np.ascontiguousarray(np.concatenate(outs, axis=0)).astype(np.float32)


# revision 23
# speedup vs baseline: 1.9303x; 1.0853x over previous
"""Trainium2 Bass kernel for nn_BasicNCAModel (neural cellular automaton).

Model (per step, 4 steps):
  y = concat([x, dwconv3x3(x, f1), dwconv3x3(x, f2)])   (reflect pad)
  dx = relu(y @ w1 + b1) @ w2
  x  = x + dx * (stoch > 0.5) * ch_mask

Kernel strategy:
  - Pure data parallel: batch 16 -> 2 samples on each of 8 NeuronCores.
  - Channel-major layout [C=32, H, W]; depthwise convs + first dense layer
    fold into a 3x3 conv with effective weights
    W_eff[dy,dx] = diag(f1[dy,dx]) @ w1[32:64] + diag(f2[dy,dx]) @ w1[64:96]
    (+ w1[0:32] at the center tap). K = 3 vertically shifted copies of x
    stacked on partitions; horizontal taps are free-dim AP offsets.
  - Mixed precision: the two outer horizontal taps (only ~15% of the dx
    variance) form the 2 k-tiles of ONE fp8e4 DoubleRow matmul per output
    half (overlapping stride-2 moving AP over an fp8 copy of the band);
    the center tap (K=97 incl. ones row for the bias) and layer 2 run in
    fp16 to keep quantization error ~1e-2. 6 PE passes of 512 moving rows
    per 512-pixel tile (vs 8 all-fp16 passes).
  - The fire mask is precomputed 0/1 fp8 on host [H, W] and replicated 32x
    on device into [H/2, 32, 2, W] via DRAM->DRAM DMAs; its (row-pair,
    channel) -> partition map is affine so one HWDGE DMA loads a whole
    band's mask [32, BR*W] with no on-chip broadcast.
  - Relu+bias splits between the scalar engine (cols [0:RELU_ACT]) and DVE
    (tensor_scalar_max, cols [RELU_ACT:1024]), both writing fp8.
  - Residual: dxm = mask * dx on DVE (PSUM source) into a band-level fp16
    tile; the state never passes through compute engines - the old band is
    DRAM->DRAM precopied to dst and dxm is accumulated into it by a SWDGE
    read-modify-write DMA (interior + pad columns).
  - Software pipelining: layer 2 runs at a 2-tile lag behind layer 1 (the
    relu latency never stalls the PE), and band i+1's loads are emitted
    before band i's accum stores so SWDGE prefetch crosses band bounds.
  - State x is fp16 DRAM, column-padded [C, H, W+2] with reflect columns
    stored in the pads (band loads contiguous). Ping-pong across steps.
"""

import numpy as np
import ml_dtypes
from contextlib import ExitStack

import concourse.bacc as bacc
import concourse.tile as tile
from concourse import mybir
from concourse.ap import AP
from concourse.bass_utils import run_bass_kernel_spmd

F32 = mybir.dt.float32
F16 = mybir.dt.float16
F8 = mybir.dt.float8e4
AF = mybir.ActivationFunctionType
OP = mybir.AluOpType
DRMODE = mybir.MatmulPerfMode.DoubleRow

B, C, H, W = 16, 32, 256, 256
IMG = 3
FIRE = 0.5
NCORES = 8
BPC = B // NCORES          # samples per core = 2
BR = 16                    # band rows
NB = H // BR               # bands per sample = 16
ROWS_PER_TILE = 2          # 2 rows x 256 cols = 512-pixel matmul tiles
TPB = BR // ROWS_PER_TILE  # tiles per band = 8
NSTEP = 4
WP = W + 2                 # padded row length 258

RELU_ACT = 704             # relu split: scalar engine cols [0:RELU_ACT]


def _seg_rows(r0: int, dy: int):
    """Contiguous (src_row, dst_row, n) segments for one vertical copy,
    with reflect handling at the image top/bottom (reflect: -1->1, 256->254)."""
    rows = [r0 + dy + i for i in range(BR)]
    refl = [(-r if r < 0 else (2 * (H - 1) - r if r > H - 1 else r)) for r in rows]
    segs = []
    i = 0
    while i < BR:
        j = i + 1
        while j < BR and refl[j] == refl[i] + (j - i):
            j += 1
        segs.append((refl[i], i, j - i))
        i = j
    return segs


def _build():
    nc = bacc.Bacc("TRN2", target_bir_lowering=False, debug=False,
                   num_devices=NCORES)
    xin = nc.dram_tensor("xin", [BPC, C, H, WP], F16, kind="ExternalInput").ap()
    mask8 = nc.dram_tensor("mask8", [NSTEP, BPC, H, W], F8,
                           kind="ExternalInput").ap()
    wmp = nc.dram_tensor("wmp", [96, 512], F8, kind="ExternalInput").ap()
    w0 = nc.dram_tensor("w0", [97, 256], F16, kind="ExternalInput").ap()
    w2h = nc.dram_tensor("w2h", [128, 64], F16, kind="ExternalInput").ap()
    yout = nc.dram_tensor("y", [BPC, C, H, WP], F16, kind="ExternalOutput").ap()

    with tile.TileContext(nc) as tc, ExitStack() as ctx:
        dram = ctx.enter_context(tc.tile_pool(name="dram", bufs=1, space="DRAM"))
        xA = dram.tile([BPC, C, H, WP], F16, name="xA")
        xB = dram.tile([BPC, C, H, WP], F16, name="xB")
        # mask replicated 32x: [step, s, row-pair, channel-copy, 2, W]
        mrep = dram.tile([NSTEP, BPC, H // 2, 32, 2, W], F8, name="mrep")

        # ---- replicate the compact mask to all 32 channel slots ----
        for step in range(NSTEP):
            for s in range(BPC):
                msrc = mask8[step, s].rearrange("(p two) w -> p two w", two=2)
                for c in range(32):
                    nc.sync.dma_start(mrep[step, s, :, c], msrc)

        wpool = ctx.enter_context(tc.tile_pool(name="wpool", bufs=1))
        wmpt = wpool.tile([96, 512], F8, name="wmpt")
        w0t = wpool.tile([97, 256], F16, name="w0t")
        w2t = wpool.tile([128, 64], F16, name="w2t")
        nc.sync.dma_start(wmpt[:], wmp)
        nc.sync.dma_start(w0t[:], w0)
        nc.sync.dma_start(w2t[:], w2h)

        xt_pool = ctx.enter_context(tc.tile_pool(name="xt", bufs=4))
        ms_pool = ctx.enter_context(tc.tile_pool(name="ms", bufs=3))
        dxm_pool = ctx.enter_context(tc.tile_pool(name="dxm", bufs=2))
        hs_pool = ctx.enter_context(tc.tile_pool(name="hs", bufs=3))
        hp_pool = ctx.enter_context(tc.tile_pool(name="hp", bufs=3, space="PSUM"))
        dxp_pool = ctx.enter_context(tc.tile_pool(name="dxp", bufs=2, space="PSUM"))

        # fp8 copy of the band for the DoubleRow outer-tap pass
        xt8_pool = ctx.enter_context(tc.tile_pool(name="xt8", bufs=4))
        # manual 4-buffer rotation for the fp16 copy so the ones row (bias)
        # is primed once per buffer instead of per band
        xts = [xt_pool.tile([97, BR * WP], F16, name=f"xt{i}")
               for i in range(4)]
        for xt in xts:
            nc.gpsimd.memset(xt[96:97, :], 1.0)

        srcs = [xin, xA[:], xB[:], xA[:]]
        dsts = [xA[:], xB[:], xA[:], yout]
        bands = [(step, s, b) for step in range(NSTEP)
                 for s in range(BPC) for b in range(NB)]
        state = {}  # band index -> dict of live tiles

        def emit_loads(i):
            step, s, b = bands[i]
            src, dst = srcs[step], dsts[step]
            r0 = b * BR
            dst_band = dst[s, :, r0:r0 + BR, :]
            # precopy old state band into dst (DRAM->DRAM); dxm will be
            # accumulated into it at band end
            nc.sync.dma_start(
                dst_band.rearrange("p r c -> p (r c)"),
                src[s, :, r0:r0 + BR, :].rearrange("p r c -> p (r c)"))
            # ---- load: 3 vertically shifted copies of the band.
            # partition groups: 0-31 dy=0, 32-63 dy=-1, 64-95 dy=+1.
            # fp16 copy via HWDGE (center tap); fp8 copy via SWDGE cast in
            # flight (DoubleRow outer taps). Reflect columns are already
            # stored in the DRAM pads.
            xt = xts[i % 4]
            xtr = xt[:].rearrange("p (r c) -> p r c", c=WP)
            xt8 = xt8_pool.tile([96, BR * WP], F8)
            xt8r = xt8[:].rearrange("p (r c) -> p r c", c=WP)
            for gi, dy in enumerate((0, -1, 1)):
                p0 = gi * 32
                for (sr, dr, n) in _seg_rows(r0, dy):
                    nc.sync.dma_start(xtr[p0:p0 + 32, dr:dr + n, :],
                                      src[s, :, sr:sr + n, :])
                    nc.gpsimd.dma_start(xt8r[p0:p0 + 32, dr:dr + n, :],
                                        src[s, :, sr:sr + n, :])
            # ---- band fire mask [32, BR*W] via one affine DMA ----
            ms = ms_pool.tile([32, BR * W], F8)
            rp0 = r0 // 2
            nc.sync.dma_start(ms[:], mrep[step, s, rp0:rp0 + TPB]
                              .rearrange("a b c d -> b a (c d)"))
            state[i] = dict(xt=xt, xtr=xtr, xt8=xt8, ms=ms,
                            dst_band=dst_band)

        def emit_compute(i):
            st = state.pop(i)
            xtr, xt8, ms = st["xtr"], st["xt8"], st["ms"]
            dst_band = st["dst_band"]
            dxm = dxm_pool.tile([32, BR * W], F16)
            xbase = xt8[:]
            pstride = xbase.ap[0][0]

            # software pipeline: layer 2 + mask of tile t-2 are emitted
            # after layer 1 + relu of tile t so the relu latency never
            # stalls the PE stream
            pend = []  # [(hs, t), ...] awaiting layer 2
            for t in range(TPB + 2):
                if t < TPB:
                    rt = t * ROWS_PER_TILE
                    # ---- layer 1: DR (taps -1,+1) + center, x2 halves ----
                    hp = hp_pool.tile([128, 1024], F32)
                    rhs_mp = AP(
                        tensor=xbase.tensor,
                        offset=xbase.offset + rt * WP,
                        ap=[[pstride, 96], [2, 2],
                            [WP, ROWS_PER_TILE], [1, W]])
                    for h in range(2):
                        out = hp[:, h * 512:(h + 1) * 512]
                        lhs_mp = (wmpt[:, h * 256:(h + 1) * 256]
                                  .rearrange("p (i m) -> p i m", i=2))
                        nc.tensor.matmul(out, lhs_mp, rhs_mp,
                                         start=True, stop=False,
                                         perf_mode=DRMODE)
                        nc.tensor.matmul(
                            out, w0t[:, h * 128:(h + 1) * 128],
                            xtr[0:97, rt:rt + ROWS_PER_TILE, 1:W + 1],
                            start=False, stop=True)
                    # ---- relu (bias via ones row) -> fp16, split
                    # between scalar engine and DVE ----
                    hs = hs_pool.tile([128, 1024], F16)
                    nc.scalar.activation(hs[:, 0:RELU_ACT],
                                         hp[:, 0:RELU_ACT], AF.Relu)
                    nc.vector.tensor_scalar_max(
                        hs[:, RELU_ACT:1024], hp[:, RELU_ACT:1024], 0.0)
                    pend.append((hs, t))
                if len(pend) <= (2 if t < TPB else 0):
                    continue
                hs_p, tp = pend.pop(0)
                # ---- layer 2: dx = h @ w2, 2 fp16 matmuls ----
                dxp = dxp_pool.tile([32, 512], F32)
                nc.tensor.matmul(dxp[:], w2t[:, 0:32], hs_p[:, 0:512],
                                 start=True, stop=False)
                nc.tensor.matmul(dxp[:], w2t[:, 32:64], hs_p[:, 512:1024],
                                 start=False, stop=True)
                # ---- dxm = mask * dx (fp16) ----
                csl = slice(tp * 512, (tp + 1) * 512)
                nc.vector.tensor_tensor(dxm[:, csl], ms[:, csl], dxp[:],
                                        op=OP.mult)

            dxmr = dxm[:].rearrange("p (r c) -> p r c", c=W)
            # interior: dst[., r, 1+w] += dxm[., r, w]  (SWDGE RMW)
            nc.gpsimd.dma_start(dst_band[:, :, 1:W + 1], dxmr,
                                accum_op=OP.add)
            # reflect pads: dst col0 += dxm col1; col257 += dxm col254
            nc.gpsimd.dma_start(
                dst_band[:, :, 0:1].rearrange("p r o -> p (r o)"),
                dxmr[:, :, 1:2].rearrange("p r o -> p (r o)"),
                accum_op=OP.add)
            nc.gpsimd.dma_start(
                dst_band[:, :, WP - 1:WP].rearrange("p r o -> p (r o)"),
                dxmr[:, :, 254:255].rearrange("p r o -> p (r o)"),
                accum_op=OP.add)

        # software-pipelined bands: band i+1's loads are emitted (and its
        # SWDGE triggers queued) before band i's accum stores, so the next
        # band's data is in flight while the current band computes
        emit_loads(0)
        for i in range(len(bands)):
            if i + 1 < len(bands):
                emit_loads(i + 1)
            emit_compute(i)
    nc.compile()
    return nc


_NC_CACHE = None


def _get_nc():
    global _NC_CACHE
    if _NC_CACHE is None:
        _NC_CACHE = _build()
    return _NC_CACHE


def _make_in_maps(x, f1, f2, w1, b1, w2, stoch):
    f1 = np.asarray(f1, np.float64)[:, :, 0, :]   # [3,3,32]
    f2 = np.asarray(f2, np.float64)[:, :, 0, :]
    w1 = np.asarray(w1, np.float64)               # [96,256]
    b1 = np.asarray(b1, np.float64)               # [256]
    w2 = np.asarray(w2, np.float64).copy()        # [256,32]
    w2[:, :IMG] = 0.0                             # ch_mask folded into w2

    # W_eff[dy,dx][c,:] = f1[dy,dx,c]*w1[32+c,:] + f2[dy,dx,c]*w1[64+c,:]
    #                     (+ w1[c,:] at the center tap)
    weff = (f1[:, :, :, None] * w1[None, None, 32:64, :]
            + f2[:, :, :, None] * w1[None, None, 64:96, :])   # [3,3,32,256]
    weff[1, 1] += w1[0:32, :]

    def col(dxi):  # stack the 3 vertical taps along K for horizontal tap dxi
        # row order matches xt partition groups: dy=0, dy=-1, dy=+1
        return np.concatenate([weff[1, dxi], weff[0, dxi], weff[2, dxi]], axis=0)

    F8NP = ml_dtypes.float8_e4m3
    wm, wpm = col(0), col(2)
    # DR stationary per half: [wm_half | wp_half] along the free dim
    wmp = np.concatenate([wm[:, 0:128], wpm[:, 0:128],
                          wm[:, 128:256], wpm[:, 128:256]], axis=1).astype(F8NP)
    w0 = np.concatenate([col(1), b1[None, :]], axis=0).astype(np.float16)
    w2h = np.concatenate([w2[0:128, :], w2[128:256, :]],
                         axis=1).astype(np.float16)

    x = np.asarray(x, np.float32)
    m8 = (np.asarray(stoch, np.float64) > FIRE).astype(F8NP)
    in_maps = []
    for i in range(NCORES):
        xi = np.transpose(x[i * BPC:(i + 1) * BPC], (0, 3, 1, 2))  # [2,32,H,W]
        xpad = np.zeros((BPC, C, H, WP), np.float16)
        xpad[:, :, :, 1:W + 1] = xi
        xpad[:, :, :, 0] = xi[:, :, :, 1]        # reflect col pads
        xpad[:, :, :, WP - 1] = xi[:, :, :, W - 2]
        mi = np.ascontiguousarray(m8[:, i * BPC:(i + 1) * BPC, :, :, 0])
        in_maps.append({"xin": xpad, "mask8": mi, "wmp": wmp, "w0": w0,
                        "w2h": w2h})
    return in_maps


def kernel(x, f1, f2, w1, b1, w2, stoch, steps):
    assert int(steps) == NSTEP, f"kernel compiled for {NSTEP} steps, got {steps}"
    nc = _get_nc()
    in_maps = _make_in_maps(x, f1, f2, w1, b1, w2, stoch)
    res = run_bass_kernel_spmd(nc, in_maps, core_ids=list(range(NCORES)))
    outs = []
    for i in range(NCORES):
        yi = res.results[i]["y"][:, :, :, 1:W + 1]     # strip col pads
        outs.append(np.transpose(yi, (0, 2, 3, 1)))    # -> [2,256,256,32]
    return np.ascontiguousarray(np.concatenate(outs, axis=0)).astype(np.float32)


# revision 32
# speedup vs baseline: 1.9680x; 1.0195x over previous
"""Trainium2 Bass kernel for nn_BasicNCAModel (neural cellular automaton).

Model (per step, 4 steps):
  y = concat([x, dwconv3x3(x, f1), dwconv3x3(x, f2)])   (reflect pad)
  dx = relu(y @ w1 + b1) @ w2
  x  = x + dx * (stoch > 0.5) * ch_mask

Kernel strategy:
  - Pure data parallel: batch 16 -> 2 samples on each of 8 NeuronCores.
  - Channel-major layout [C=32, H, W]; depthwise convs + first dense layer
    fold into a 3x3 conv with effective weights
    W_eff[dy,dx] = diag(f1[dy,dx]) @ w1[32:64] + diag(f2[dy,dx]) @ w1[64:96]
    (+ w1[0:32] at the center tap). K = 3 vertically shifted copies of x
    stacked on partitions; horizontal taps are free-dim AP offsets.
  - Mixed precision: the two outer horizontal taps (only ~15% of the dx
    variance) form the 2 k-tiles of ONE fp8e4 DoubleRow matmul per output
    half (overlapping stride-2 moving AP over an fp8 copy of the band);
    the center tap (K=97 incl. ones row for the bias) and layer 2 run in
    fp16 to keep quantization error ~1e-2. 6 PE passes of 512 moving rows
    per 512-pixel tile (vs 8 all-fp16 passes).
  - The fire mask is precomputed 0/1 fp8 on host [H, W] and replicated 32x
    on device into [H/2, 32, 2, W] via DRAM->DRAM DMAs; its (row-pair,
    channel) -> partition map is affine so one HWDGE DMA loads a whole
    band's mask [32, BR*W] with no on-chip broadcast.
  - Relu+bias splits between the scalar engine (cols [0:RELU_ACT]) and DVE
    (tensor_scalar_max, cols [RELU_ACT:1024]), both writing fp8.
  - Residual: dxm = mask * dx on DVE (PSUM source), then xn = dxm + xc in
    fp16 on DVE (2x mode); relu runs fully on the scalar engine. This
    keeps the SWDGE queue for the xt8 cast-load only (HWDGE carries the
    fp16 state traffic), so neither DMA path saturates.
  - Software pipelining: layer 2 runs at a 2-tile lag behind layer 1 (the
    relu latency never stalls the PE), and band i+1's loads are emitted
    before band i's accum stores so SWDGE prefetch crosses band bounds.
  - State x is fp16 DRAM, column-padded [C, H, W+2] with reflect columns
    stored in the pads (band loads contiguous). Ping-pong across steps.
"""

import numpy as np
import ml_dtypes
from contextlib import ExitStack

import concourse.bacc as bacc
import concourse.tile as tile
from concourse import mybir
from concourse.ap import AP
from concourse.bass_utils import run_bass_kernel_spmd

F32 = mybir.dt.float32
F16 = mybir.dt.float16
F8 = mybir.dt.float8e4
AF = mybir.ActivationFunctionType
OP = mybir.AluOpType
DRMODE = mybir.MatmulPerfMode.DoubleRow

B, C, H, W = 16, 32, 256, 256
IMG = 3
FIRE = 0.5
NCORES = 8
BPC = B // NCORES          # samples per core = 2
BR = 16                    # band rows
NB = H // BR               # bands per sample = 16
ROWS_PER_TILE = 2          # 2 rows x 256 cols = 512-pixel matmul tiles
TPB = BR // ROWS_PER_TILE  # tiles per band = 8
NSTEP = 4
WP = W + 2                 # padded row length 258

RELU_ACT = 1024            # relu split: scalar engine cols [0:RELU_ACT]


def _seg_rows(r0: int, dy: int):
    """Contiguous (src_row, dst_row, n) segments for one vertical copy,
    with reflect handling at the image top/bottom (reflect: -1->1, 256->254)."""
    rows = [r0 + dy + i for i in range(BR)]
    refl = [(-r if r < 0 else (2 * (H - 1) - r if r > H - 1 else r)) for r in rows]
    segs = []
    i = 0
    while i < BR:
        j = i + 1
        while j < BR and refl[j] == refl[i] + (j - i):
            j += 1
        segs.append((refl[i], i, j - i))
        i = j
    return segs


def _build():
    nc = bacc.Bacc("TRN2", target_bir_lowering=False, debug=False,
                   num_devices=NCORES)
    xin = nc.dram_tensor("xin", [BPC, C, H, WP], F16, kind="ExternalInput").ap()
    mask8 = nc.dram_tensor("mask8", [NSTEP, BPC, H, W], F8,
                           kind="ExternalInput").ap()
    wmp = nc.dram_tensor("wmp", [96, 512], F8, kind="ExternalInput").ap()
    w0 = nc.dram_tensor("w0", [97, 256], F16, kind="ExternalInput").ap()
    w2h = nc.dram_tensor("w2h", [128, 64], F16, kind="ExternalInput").ap()
    yout = nc.dram_tensor("y", [BPC, C, H, WP], F16, kind="ExternalOutput").ap()

    with tile.TileContext(nc) as tc, ExitStack() as ctx:
        dram = ctx.enter_context(tc.tile_pool(name="dram", bufs=1, space="DRAM"))
        xA = dram.tile([BPC, C, H, WP], F16, name="xA")
        xB = dram.tile([BPC, C, H, WP], F16, name="xB")
        # mask replicated 32x: [step, s, row-pair, channel-copy, 2, W]
        mrep = dram.tile([NSTEP, BPC, H // 2, 32, 2, W], F8, name="mrep")

        # ---- replicate the compact mask to all 32 channel slots ----
        for step in range(NSTEP):
            for s in range(BPC):
                msrc = mask8[step, s].rearrange("(p two) w -> p two w", two=2)
                for c in range(32):
                    nc.sync.dma_start(mrep[step, s, :, c], msrc)

        wpool = ctx.enter_context(tc.tile_pool(name="wpool", bufs=1))
        wmpt = wpool.tile([96, 512], F8, name="wmpt")
        w0t = wpool.tile([97, 256], F16, name="w0t")
        w2t = wpool.tile([128, 64], F16, name="w2t")
        nc.sync.dma_start(wmpt[:], wmp)
        nc.sync.dma_start(w0t[:], w0)
        nc.sync.dma_start(w2t[:], w2h)

        xt_pool = ctx.enter_context(tc.tile_pool(name="xt", bufs=1))
        ms_pool = ctx.enter_context(tc.tile_pool(name="ms", bufs=3))
        dxm_pool = ctx.enter_context(tc.tile_pool(name="dxm", bufs=2))
        hs_pool = ctx.enter_context(tc.tile_pool(name="hs", bufs=3))
        hp_pool = ctx.enter_context(tc.tile_pool(name="hp", bufs=3, space="PSUM"))
        dxp_pool = ctx.enter_context(tc.tile_pool(name="dxp", bufs=2, space="PSUM"))
        xc_pool = ctx.enter_context(tc.tile_pool(name="xc", bufs=3))
        xn_pool = ctx.enter_context(tc.tile_pool(name="xn", bufs=2))

        # fp8 copy of the band for the DoubleRow outer-tap pass
        xt8_pool = ctx.enter_context(tc.tile_pool(name="xt8", bufs=4))
        # manual 4-buffer rotation for the fp16 copy so the ones row (bias)
        # is primed once per buffer instead of per band
        xts = [xt_pool.tile([97, BR * WP], F16, name=f"xt{i}")
               for i in range(4)]
        for xt in xts:
            nc.gpsimd.memset(xt[96:97, :], 1.0)

        srcs = [xin, xA[:], xB[:], xA[:]]
        dsts = [xA[:], xB[:], xA[:], yout]
        bands = [(step, s, b) for step in range(NSTEP)
                 for s in range(BPC) for b in range(NB)]
        state = {}  # band index -> dict of live tiles

        def emit_loads(i):
            step, s, b = bands[i]
            src, dst = srcs[step], dsts[step]
            r0 = b * BR
            dst_band = dst[s, :, r0:r0 + BR, :]
            # ---- load: 3 vertically shifted copies of the band.
            # partition groups: 0-31 dy=0, 32-63 dy=-1, 64-95 dy=+1.
            # fp16 copy via HWDGE (center tap); fp8 copy via SWDGE cast in
            # flight (DoubleRow outer taps). Reflect columns are already
            # stored in the DRAM pads.
            xt = xts[i % 4]
            xtr = xt[:].rearrange("p (r c) -> p r c", c=WP)
            xt8 = xt8_pool.tile([96, BR * WP], F8)
            xt8r = xt8[:].rearrange("p (r c) -> p r c", c=WP)
            for gi, dy in enumerate((0, -1, 1)):
                p0 = gi * 32
                for (sr, dr, n) in _seg_rows(r0, dy):
                    nc.sync.dma_start(xtr[p0:p0 + 32, dr:dr + n, :],
                                      src[s, :, sr:sr + n, :])
                    nc.gpsimd.dma_start(xt8r[p0:p0 + 32, dr:dr + n, :],
                                        src[s, :, sr:sr + n, :])
            # ---- band fire mask [32, BR*W] via one affine DMA ----
            ms = ms_pool.tile([32, BR * W], F8)
            rp0 = r0 // 2
            nc.sync.dma_start(ms[:], mrep[step, s, rp0:rp0 + TPB]
                              .rearrange("a b c d -> b a (c d)"))
            # fp16 copy of the band for the residual add
            xc = xc_pool.tile([32, BR * WP], F16)
            nc.sync.dma_start(xc[:], src[s, :, r0:r0 + BR, :]
                              .rearrange("p r c -> p (r c)"))
            state[i] = dict(xt=xt, xtr=xtr, xt8=xt8, ms=ms, xc=xc,
                            dst_band=dst_band)

        def emit_compute(i):
            st = state.pop(i)
            xtr, xt8, ms = st["xtr"], st["xt8"], st["ms"]
            dst_band = st["dst_band"]
            xcr = st["xc"][:].rearrange("p (r c) -> p r c", c=WP)
            dxm = dxm_pool.tile([32, BR * W], F16)
            xn = xn_pool.tile([32, BR * WP], F16)
            xnr = xn[:].rearrange("p (r c) -> p r c", c=WP)
            xbase = xt8[:]
            pstride = xbase.ap[0][0]

            # software pipeline: layer 2 + mask of tile t-2 are emitted
            # after layer 1 + relu of tile t so the relu latency never
            # stalls the PE stream
            pend = []  # [(hs, t), ...] awaiting layer 2
            for t in range(TPB + 2):
                if t < TPB:
                    rt = t * ROWS_PER_TILE
                    # ---- layer 1: DR (taps -1,+1) + center, x2 halves ----
                    hp = hp_pool.tile([128, 1024], F32)
                    rhs_mp = AP(
                        tensor=xbase.tensor,
                        offset=xbase.offset + rt * WP,
                        ap=[[pstride, 96], [2, 2],
                            [WP, ROWS_PER_TILE], [1, W]])
                    for h in range(2):
                        out = hp[:, h * 512:(h + 1) * 512]
                        lhs_mp = (wmpt[:, h * 256:(h + 1) * 256]
                                  .rearrange("p (i m) -> p i m", i=2))
                        nc.tensor.matmul(out, lhs_mp, rhs_mp,
                                         start=True, stop=False,
                                         perf_mode=DRMODE)
                        nc.tensor.matmul(
                            out, w0t[:, h * 128:(h + 1) * 128],
                            xtr[0:97, rt:rt + ROWS_PER_TILE, 1:W + 1],
                            start=False, stop=True)
                    # ---- relu (bias via ones row) -> fp16 ----
                    hs = hs_pool.tile([128, 1024], F16)
                    nc.scalar.activation(hs[:, 0:RELU_ACT],
                                         hp[:, 0:RELU_ACT], AF.Relu)
                    if RELU_ACT < 1024:
                        nc.vector.tensor_scalar_max(
                            hs[:, RELU_ACT:1024], hp[:, RELU_ACT:1024], 0.0)
                    pend.append((hs, t))
                if len(pend) <= (2 if t < TPB else 0):
                    continue
                hs_p, tp = pend.pop(0)
                # ---- layer 2: dx = h @ w2, 2 fp16 matmuls ----
                dxp = dxp_pool.tile([32, 512], F32)
                nc.tensor.matmul(dxp[:], w2t[:, 0:32], hs_p[:, 0:512],
                                 start=True, stop=False)
                nc.tensor.matmul(dxp[:], w2t[:, 32:64], hs_p[:, 512:1024],
                                 start=False, stop=True)
                # ---- dxm = mask * dx (fp16) ----
                csl = slice(tp * 512, (tp + 1) * 512)
                nc.vector.tensor_tensor(dxm[:, csl], ms[:, csl], dxp[:],
                                        op=OP.mult)
                # ---- residual add xn = dxm + xc (fp16, DVE 2x) ----
                rp = tp * ROWS_PER_TILE
                nc.vector.tensor_add(
                    xnr[:, rp:rp + ROWS_PER_TILE, 1:W + 1],
                    dxm[:, csl].rearrange("p (r c) -> p r c", c=W),
                    xcr[:, rp:rp + ROWS_PER_TILE, 1:W + 1])

            # reflect pads then store the fp16 band (HWDGE)
            nc.vector.tensor_copy(xnr[:, :, 0:1], xnr[:, :, 2:3])
            nc.vector.tensor_copy(xnr[:, :, WP - 1:WP],
                                  xnr[:, :, WP - 3:WP - 2])
            nc.sync.dma_start(dst_band.rearrange("p r c -> p (r c)"), xn[:])

        # software-pipelined bands: band i+1's loads are emitted (and its
        # SWDGE triggers queued) before band i's accum stores, so the next
        # band's data is in flight while the current band computes
        emit_loads(0)
        for i in range(len(bands)):
            if i + 1 < len(bands):
                emit_loads(i + 1)
            emit_compute(i)
    nc.compile()
    return nc


_NC_CACHE = None


def _get_nc():
    global _NC_CACHE
    if _NC_CACHE is None:
        _NC_CACHE = _build()
    return _NC_CACHE


def _make_in_maps(x, f1, f2, w1, b1, w2, stoch):
    f1 = np.asarray(f1, np.float64)[:, :, 0, :]   # [3,3,32]
    f2 = np.asarray(f2, np.float64)[:, :, 0, :]
    w1 = np.asarray(w1, np.float64)               # [96,256]
    b1 = np.asarray(b1, np.float64)               # [256]
    w2 = np.asarray(w2, np.float64).copy()        # [256,32]
    w2[:, :IMG] = 0.0                             # ch_mask folded into w2

    # W_eff[dy,dx][c,:] = f1[dy,dx,c]*w1[32+c,:] + f2[dy,dx,c]*w1[64+c,:]
    #                     (+ w1[c,:] at the center tap)
    weff = (f1[:, :, :, None] * w1[None, None, 32:64, :]
            + f2[:, :, :, None] * w1[None, None, 64:96, :])   # [3,3,32,256]
    weff[1, 1] += w1[0:32, :]

    def col(dxi):  # stack the 3 vertical taps along K for horizontal tap dxi
        # row order matches xt partition groups: dy=0, dy=-1, dy=+1
        return np.concatenate([weff[1, dxi], weff[0, dxi], weff[2, dxi]], axis=0)

    F8NP = ml_dtypes.float8_e4m3
    wm, wpm = col(0), col(2)
    # DR stationary per half: [wm_half | wp_half] along the free dim
    wmp = np.concatenate([wm[:, 0:128], wpm[:, 0:128],
                          wm[:, 128:256], wpm[:, 128:256]], axis=1).astype(F8NP)
    w0 = np.concatenate([col(1), b1[None, :]], axis=0).astype(np.float16)
    w2h = np.concatenate([w2[0:128, :], w2[128:256, :]],
                         axis=1).astype(np.float16)

    x = np.asarray(x, np.float32)
    m8 = (np.asarray(stoch, np.float64) > FIRE).astype(F8NP)
    in_maps = []
    for i in range(NCORES):
        xi = np.transpose(x[i * BPC:(i + 1) * BPC], (0, 3, 1, 2))  # [2,32,H,W]
        xpad = np.zeros((BPC, C, H, WP), np.float16)
        xpad[:, :, :, 1:W + 1] = xi
        xpad[:, :, :, 0] = xi[:, :, :, 1]        # reflect col pads
        xpad[:, :, :, WP - 1] = xi[:, :, :, W - 2]
        mi = np.ascontiguousarray(m8[:, i * BPC:(i + 1) * BPC, :, :, 0])
        in_maps.append({"xin": xpad, "mask8": mi, "wmp": wmp, "w0": w0,
                        "w2h": w2h})
    return in_maps


def kernel(x, f1, f2, w1, b1, w2, stoch, steps):
    assert int(steps) == NSTEP, f"kernel compiled for {NSTEP} steps, got {steps}"
    nc = _get_nc()
    in_maps = _make_in_maps(x, f1, f2, w1, b1, w2, stoch)
    res = run_bass_kernel_spmd(nc, in_maps, core_ids=list(range(NCORES)))
    outs = []
    for i in range(NCORES):
        yi = res.results[i]["y"][:, :, :, 1:W + 1]     # strip col pads
        outs.append(np.transpose(yi, (0, 2, 3, 1)))    # -> [2,256,256,32]
    return np.ascontiguousarray(np.concatenate(outs, axis=0)).astype(np.float32)


# revision 36
# speedup vs baseline: 2.0517x; 1.0425x over previous
"""Trainium2 Bass kernel for nn_BasicNCAModel (neural cellular automaton).

Model (per step, 4 steps):
  y = concat([x, dwconv3x3(x, f1), dwconv3x3(x, f2)])   (reflect pad)
  dx = relu(y @ w1 + b1) @ w2
  x  = x + dx * (stoch > 0.5) * ch_mask

Kernel strategy:
  - Pure data parallel: batch 16 -> 2 samples on each of 8 NeuronCores.
  - Channel-major layout [C=32, H, W]; depthwise convs + first dense layer
    fold into a 3x3 conv with effective weights
    W_eff[dy,dx] = diag(f1[dy,dx]) @ w1[32:64] + diag(f2[dy,dx]) @ w1[64:96]
    (+ w1[0:32] at the center tap). K = 3 vertically shifted copies of x
    stacked on partitions; horizontal taps are free-dim AP offsets.
  - Mixed precision: the two outer horizontal taps (only ~15% of the dx
    variance) form the 2 k-tiles of ONE fp8e4 DoubleRow matmul per output
    half (overlapping stride-2 moving AP over an fp8 copy of the band);
    the center tap (K=97 incl. ones row for the bias) and layer 2 run in
    fp16 to keep quantization error ~1e-2. 6 PE passes of 512 moving rows
    per 512-pixel tile (vs 8 all-fp16 passes).
  - The fire mask is precomputed 0/1 fp8 on host [H, W] and replicated 32x
    on device into [H/2, 32, 2, W] via DRAM->DRAM DMAs; its (row-pair,
    channel) -> partition map is affine so one HWDGE DMA loads a whole
    band's mask [32, BR*W] with no on-chip broadcast.
  - Relu+bias splits between the scalar engine (cols [0:RELU_ACT]) and DVE
    (tensor_scalar_max, cols [RELU_ACT:1024]), both writing fp8.
  - Residual: dxm = mask * dx on DVE (PSUM source), then xn = dxm + xc in
    fp16 on DVE (2x mode); relu runs fully on the scalar engine. This
    keeps the SWDGE queue for the xt8 cast-load only (HWDGE carries the
    fp16 state traffic), so neither DMA path saturates.
  - Software pipelining: layer 2 runs at a 2-tile lag behind layer 1 (the
    relu latency never stalls the PE), and band i+1's loads are emitted
    before band i's accum stores so SWDGE prefetch crosses band bounds.
  - State x is fp16 DRAM, column-padded [C, H, W+2] with reflect columns
    stored in the pads (band loads contiguous). Ping-pong across steps.
"""

import numpy as np
import ml_dtypes
from contextlib import ExitStack

import concourse.bacc as bacc
import concourse.tile as tile
from concourse import mybir
from concourse.ap import AP
from concourse.bass_utils import run_bass_kernel_spmd

F32 = mybir.dt.float32
F16 = mybir.dt.float16
F8 = mybir.dt.float8e4
AF = mybir.ActivationFunctionType
OP = mybir.AluOpType
DRMODE = mybir.MatmulPerfMode.DoubleRow

B, C, H, W = 16, 32, 256, 256
IMG = 3
FIRE = 0.5
NCORES = 8
BPC = B // NCORES          # samples per core = 2
BR = 16                    # band rows
NB = H // BR               # bands per sample = 16
ROWS_PER_TILE = 2          # 2 rows x 256 cols = 512-pixel matmul tiles
TPB = BR // ROWS_PER_TILE  # tiles per band = 8
NSTEP = 4
WP = W + 2                 # padded row length 258

RELU_ACT = 1024            # relu split: scalar engine cols [0:RELU_ACT]


def _seg_rows(r0: int, dy: int):
    """Contiguous (src_row, dst_row, n) segments for one vertical copy,
    with reflect handling at the image top/bottom (reflect: -1->1, 256->254)."""
    rows = [r0 + dy + i for i in range(BR)]
    refl = [(-r if r < 0 else (2 * (H - 1) - r if r > H - 1 else r)) for r in rows]
    segs = []
    i = 0
    while i < BR:
        j = i + 1
        while j < BR and refl[j] == refl[i] + (j - i):
            j += 1
        segs.append((refl[i], i, j - i))
        i = j
    return segs


def _build():
    nc = bacc.Bacc("TRN2", target_bir_lowering=False, debug=False,
                   num_devices=NCORES)
    xin = nc.dram_tensor("xin", [BPC, C, H, WP], F16, kind="ExternalInput").ap()
    mask8 = nc.dram_tensor("mask8", [NSTEP, BPC, H, W], F8,
                           kind="ExternalInput").ap()
    wmp = nc.dram_tensor("wmp", [96, 512], F8, kind="ExternalInput").ap()
    w0 = nc.dram_tensor("w0", [97, 256], F16, kind="ExternalInput").ap()
    w2h = nc.dram_tensor("w2h", [128, 64], F16, kind="ExternalInput").ap()
    yout = nc.dram_tensor("y", [BPC, C, H, WP], F16, kind="ExternalOutput").ap()

    with tile.TileContext(nc) as tc, ExitStack() as ctx:
        dram = ctx.enter_context(tc.tile_pool(name="dram", bufs=1, space="DRAM"))
        xA = dram.tile([BPC, C, H, WP], F16, name="xA")
        xB = dram.tile([BPC, C, H, WP], F16, name="xB")
        # mask replicated 32x: [step, s, row-pair, channel-copy, 2, W]
        mrep = dram.tile([NSTEP, BPC, H // 2, 32, 2, W], F8, name="mrep")

        # ---- replicate the compact mask to all 32 channel slots by
        # log-doubling (6 DMAs per (step, sample) instead of 32) ----
        for step in range(NSTEP):
            for s in range(BPC):
                msrc = mask8[step, s].rearrange("(p two) w -> p two w", two=2)
                nc.sync.dma_start(mrep[step, s, :, 0], msrc)
                n = 1
                while n < 32:
                    nc.sync.dma_start(
                        mrep[step, s, :, n:2 * n]
                        .rearrange("p c two w -> p c (two w)"),
                        mrep[step, s, :, 0:n]
                        .rearrange("p c two w -> p c (two w)"))
                    n *= 2

        wpool = ctx.enter_context(tc.tile_pool(name="wpool", bufs=1))
        wmpt = wpool.tile([96, 512], F8, name="wmpt")
        w0t = wpool.tile([97, 256], F16, name="w0t")
        w2t = wpool.tile([128, 64], F16, name="w2t")
        nc.sync.dma_start(wmpt[:], wmp)
        nc.sync.dma_start(w0t[:], w0)
        nc.sync.dma_start(w2t[:], w2h)

        xt_pool = ctx.enter_context(tc.tile_pool(name="xt", bufs=1))
        ms_pool = ctx.enter_context(tc.tile_pool(name="ms", bufs=4))
        dxm_pool = ctx.enter_context(tc.tile_pool(name="dxm", bufs=2))
        hs_pool = ctx.enter_context(tc.tile_pool(name="hs", bufs=3))
        hp_pool = ctx.enter_context(tc.tile_pool(name="hp", bufs=3, space="PSUM"))
        dxp_pool = ctx.enter_context(tc.tile_pool(name="dxp", bufs=2, space="PSUM"))
        xc_pool = ctx.enter_context(tc.tile_pool(name="xc", bufs=4))
        xn_pool = ctx.enter_context(tc.tile_pool(name="xn", bufs=2))

        # fp8 copy of the band for the DoubleRow outer-tap pass
        xt8_pool = ctx.enter_context(tc.tile_pool(name="xt8", bufs=4))
        # manual 4-buffer rotation for the fp16 copy so the ones row (bias)
        # is primed once per buffer instead of per band
        xts = [xt_pool.tile([97, BR * WP], F16, name=f"xt{i}")
               for i in range(4)]
        for xt in xts:
            nc.gpsimd.memset(xt[96:97, :], 1.0)

        srcs = [xin, xA[:], xB[:], xA[:]]
        dsts = [xA[:], xB[:], xA[:], yout]
        bands = [(step, s, b) for step in range(NSTEP)
                 for s in range(BPC) for b in range(NB)]
        state = {}  # band index -> dict of live tiles

        def emit_loads(i):
            step, s, b = bands[i]
            src, dst = srcs[step], dsts[step]
            r0 = b * BR
            dst_band = dst[s, :, r0:r0 + BR, :]
            # ---- load: 3 vertically shifted copies of the band.
            # partition groups: 0-31 dy=0, 32-63 dy=-1, 64-95 dy=+1.
            # fp16 copy via HWDGE (center tap); fp8 copy via SWDGE cast in
            # flight (DoubleRow outer taps). Reflect columns are already
            # stored in the DRAM pads.
            xt = xts[i % 4]
            xtr = xt[:].rearrange("p (r c) -> p r c", c=WP)
            xt8 = xt8_pool.tile([96, BR * WP], F8)
            xt8r = xt8[:].rearrange("p (r c) -> p r c", c=WP)
            for gi, dy in enumerate((0, -1, 1)):
                p0 = gi * 32
                for (sr, dr, n) in _seg_rows(r0, dy):
                    nc.sync.dma_start(xtr[p0:p0 + 32, dr:dr + n, :],
                                      src[s, :, sr:sr + n, :])
                    nc.gpsimd.dma_start(xt8r[p0:p0 + 32, dr:dr + n, :],
                                        src[s, :, sr:sr + n, :])
            # ---- band fire mask [32, BR*W] via one affine DMA ----
            ms = ms_pool.tile([32, BR * W], F8)
            rp0 = r0 // 2
            nc.sync.dma_start(ms[:], mrep[step, s, rp0:rp0 + TPB]
                              .rearrange("a b c d -> b a (c d)"))
            # fp16 copy of the band for the residual add
            xc = xc_pool.tile([32, BR * WP], F16)
            nc.sync.dma_start(xc[:], src[s, :, r0:r0 + BR, :]
                              .rearrange("p r c -> p (r c)"))
            state[i] = dict(xt=xt, xtr=xtr, xt8=xt8, ms=ms, xc=xc,
                            dst_band=dst_band)

        def emit_compute(i):
            st = state.pop(i)
            xtr, xt8, ms = st["xtr"], st["xt8"], st["ms"]
            dst_band = st["dst_band"]
            xcr = st["xc"][:].rearrange("p (r c) -> p r c", c=WP)
            dxm = dxm_pool.tile([32, BR * W], F16)
            xn = xn_pool.tile([32, BR * WP], F16)
            xnr = xn[:].rearrange("p (r c) -> p r c", c=WP)
            xbase = xt8[:]
            pstride = xbase.ap[0][0]

            # software pipeline: layer 2 + mask of tile t-2 are emitted
            # after layer 1 + relu of tile t so the relu latency never
            # stalls the PE stream
            pend = []  # [(hs, t), ...] awaiting layer 2
            for t in range(TPB + 2):
                if t < TPB:
                    rt = t * ROWS_PER_TILE
                    # ---- layer 1: DR (taps -1,+1) + center, x2 halves ----
                    hp = hp_pool.tile([128, 1024], F32)
                    rhs_mp = AP(
                        tensor=xbase.tensor,
                        offset=xbase.offset + rt * WP,
                        ap=[[pstride, 96], [2, 2],
                            [WP, ROWS_PER_TILE], [1, W]])
                    for h in range(2):
                        out = hp[:, h * 512:(h + 1) * 512]
                        lhs_mp = (wmpt[:, h * 256:(h + 1) * 256]
                                  .rearrange("p (i m) -> p i m", i=2))
                        nc.tensor.matmul(out, lhs_mp, rhs_mp,
                                         start=True, stop=False,
                                         perf_mode=DRMODE)
                        nc.tensor.matmul(
                            out, w0t[:, h * 128:(h + 1) * 128],
                            xtr[0:97, rt:rt + ROWS_PER_TILE, 1:W + 1],
                            start=False, stop=True)
                    # ---- relu (bias via ones row) -> fp16 ----
                    hs = hs_pool.tile([128, 1024], F16)
                    nc.scalar.activation(hs[:, 0:RELU_ACT],
                                         hp[:, 0:RELU_ACT], AF.Relu)
                    if RELU_ACT < 1024:
                        nc.vector.tensor_scalar_max(
                            hs[:, RELU_ACT:1024], hp[:, RELU_ACT:1024], 0.0)
                    pend.append((hs, t))
                if len(pend) <= (2 if t < TPB else 0):
                    continue
                hs_p, tp = pend.pop(0)
                # ---- layer 2: dx = h @ w2, 2 fp16 matmuls ----
                dxp = dxp_pool.tile([32, 512], F32)
                nc.tensor.matmul(dxp[:], w2t[:, 0:32], hs_p[:, 0:512],
                                 start=True, stop=False)
                nc.tensor.matmul(dxp[:], w2t[:, 32:64], hs_p[:, 512:1024],
                                 start=False, stop=True)
                # ---- dxm = mask * dx (fp16) ----
                csl = slice(tp * 512, (tp + 1) * 512)
                nc.vector.tensor_tensor(dxm[:, csl], ms[:, csl], dxp[:],
                                        op=OP.mult)
                # ---- residual add xn = dxm + xc (fp16, DVE 2x) ----
                rp = tp * ROWS_PER_TILE
                nc.vector.tensor_add(
                    xnr[:, rp:rp + ROWS_PER_TILE, 1:W + 1],
                    dxm[:, csl].rearrange("p (r c) -> p r c", c=W),
                    xcr[:, rp:rp + ROWS_PER_TILE, 1:W + 1])

            # reflect pads then store the fp16 band (HWDGE)
            nc.vector.tensor_copy(xnr[:, :, 0:1], xnr[:, :, 2:3])
            nc.vector.tensor_copy(xnr[:, :, WP - 1:WP],
                                  xnr[:, :, WP - 3:WP - 2])
            nc.sync.dma_start(dst_band.rearrange("p r c -> p (r c)"), xn[:])

        # software-pipelined bands with 2-band prefetch: loads for band i+2
        # are queued before band i's store (which blocks the HWDGE queue
        # until band i's compute finishes), so load data always has two
        # band periods of lead time
        emit_loads(0)
        emit_loads(1)
        for i in range(len(bands)):
            if i + 2 < len(bands):
                emit_loads(i + 2)
            emit_compute(i)
    nc.compile()
    return nc


_NC_CACHE = None


def _get_nc():
    global _NC_CACHE
    if _NC_CACHE is None:
        _NC_CACHE = _build()
    return _NC_CACHE


def _make_in_maps(x, f1, f2, w1, b1, w2, stoch):
    f1 = np.asarray(f1, np.float64)[:, :, 0, :]   # [3,3,32]
    f2 = np.asarray(f2, np.float64)[:, :, 0, :]
    w1 = np.asarray(w1, np.float64)               # [96,256]
    b1 = np.asarray(b1, np.float64)               # [256]
    w2 = np.asarray(w2, np.float64).copy()        # [256,32]
    w2[:, :IMG] = 0.0                             # ch_mask folded into w2

    # W_eff[dy,dx][c,:] = f1[dy,dx,c]*w1[32+c,:] + f2[dy,dx,c]*w1[64+c,:]
    #                     (+ w1[c,:] at the center tap)
    weff = (f1[:, :, :, None] * w1[None, None, 32:64, :]
            + f2[:, :, :, None] * w1[None, None, 64:96, :])   # [3,3,32,256]
    weff[1, 1] += w1[0:32, :]

    def col(dxi):  # stack the 3 vertical taps along K for horizontal tap dxi
        # row order matches xt partition groups: dy=0, dy=-1, dy=+1
        return np.concatenate([weff[1, dxi], weff[0, dxi], weff[2, dxi]], axis=0)

    F8NP = ml_dtypes.float8_e4m3
    wm, wpm = col(0), col(2)
    # DR stationary per half: [wm_half | wp_half] along the free dim
    wmp = np.concatenate([wm[:, 0:128], wpm[:, 0:128],
                          wm[:, 128:256], wpm[:, 128:256]], axis=1).astype(F8NP)
    w0 = np.concatenate([col(1), b1[None, :]], axis=0).astype(np.float16)
    w2h = np.concatenate([w2[0:128, :], w2[128:256, :]],
                         axis=1).astype(np.float16)

    x = np.asarray(x, np.float32)
    m8 = (np.asarray(stoch, np.float64) > FIRE).astype(F8NP)
    in_maps = []
    for i in range(NCORES):
        xi = np.transpose(x[i * BPC:(i + 1) * BPC], (0, 3, 1, 2))  # [2,32,H,W]
        xpad = np.zeros((BPC, C, H, WP), np.float16)
        xpad[:, :, :, 1:W + 1] = xi
        xpad[:, :, :, 0] = xi[:, :, :, 1]        # reflect col pads
        xpad[:, :, :, WP - 1] = xi[:, :, :, W - 2]
        mi = np.ascontiguousarray(m8[:, i * BPC:(i + 1) * BPC, :, :, 0])
        in_maps.append({"xin": xpad, "mask8": mi, "wmp": wmp, "w0": w0,
                        "w2h": w2h})
    return in_maps


def kernel(x, f1, f2, w1, b1, w2, stoch, steps):
    assert int(steps) == NSTEP, f"kernel compiled for {NSTEP} steps, got {steps}"
    nc = _get_nc()
    in_maps = _make_in_maps(x, f1, f2, w1, b1, w2, stoch)
    res = run_bass_kernel_spmd(nc, in_maps, core_ids=list(range(NCORES)))
    outs = []
    for i in range(NCORES):
        yi = res.results[i]["y"][:, :, :, 1:W + 1]     # strip col pads
        outs.append(np.transpose(yi, (0, 2, 3, 1)))    # -> [2,256,256,32]
    return np.ascontiguousarray(np.concatenate(outs, axis=0)).astype(np.float32)
